# revision 27
# baseline (speedup 1.0000x reference)
"""Trainium2 Bass kernel for an ExGRU cell (GRU + output projection).

Reference computation (per batch row, B=8192, IN=1024, H=1024, OUT=512):
    xh      = concat(x, h)                  # [B, IN+H]
    z       = sigmoid(xh @ W_z.T + b_z)     # [B, H]
    r       = sigmoid(xh @ W_r.T + b_r)     # [B, H]
    xrh     = concat(x, r * h)
    h_tilde = tanh(xrh @ W_h.T + b_h)       # [B, H]
    hidden  = (1 - z) * h + z * h_tilde     # [B, H]
    output  = hidden @ W_o.T + b_o          # [B, OUT]
    return (output, hidden)

Strategy: data-parallel over the batch dim — each of the 8 cores gets 1024
batch rows; weights are replicated. Zero collectives. Everything on-device is
kept in a TRANSPOSED layout ([feature, batch]) so the contraction dim is always
on SBUF partitions and no on-device transposes are ever needed:

  - host pre-transposes x, h (batch → columns) and pre-tiles the weights,
  - gate outputs come out of the PE as z^T/r^T/h_tilde^T [H, B_loc] with the
    hidden dim on partitions, which is exactly the layout the candidate matmul
    and output projection need as their streaming operand,
  - host transposes the two outputs back after the gather.

Precision/speed mix (PE-bound kernel, so matmul dtype sets the roofline):
  - z-gate + candidate matmuls: fp32r (1 cycle/row at N=512, 4x faster than
    strict fp32, ~1e-4 accuracy), accumulating K=2048 over 16 psum steps;
  - r-gate: fp8e4m3 with DoubleRow (2 k-tiles per matmul, 2x PE rate).
    Weights pre-scaled by 64 into fp8's normal range; the sigmoid's
    activation scale divides it back out. The r-gate tolerates fp8 because
    its error is damped by the sigmoid slope (<=0.25) and then again through
    the candidate matmul (~1e-3 contribution to hidden);
  - output projection: bf16 (same PE rate as fp32r; frees SBUF so the whole
    fp8 phase-A working set can preload ahead of the f32r bulk).
Measured end-to-end error vs the fp32 reference: ~2.4e-3 relative.

Phases: A) r-gate + rh=sigmoid()*h, B) z-gate + candidate + hidden update,
C) output projection. All weight slabs stream from HBM double-buffered under
the matmuls; x/h/rh/hnew stay SBUF-resident.
"""

import numpy as np

import concourse.mybir as mybir
import concourse.tile as tile
from concourse import bacc
from concourse.bass_utils import run_bass_kernel_spmd

# Problem dims (hardcoded — kernel.py must be self-contained).
B, IN, H, OUT = 8192, 1024, 1024, 512
NCORES = 8
BL = B // NCORES      # 1024 local batch columns per core
K = IN + H            # 2048 gate contraction dim
P = 128               # SBUF partitions
KT = K // P           # 16 k-tiles per gate matmul
XT_ = IN // P         # 8 k-tiles of x
HT_ = H // P          # 8 k-tiles of h / j-tiles of the hidden dim
OT_ = OUT // P        # 4 o-tiles of the output projection
NB = BL // 512        # 2 batch chunks of 512 (fp32 moving-operand max)

F32 = mybir.dt.float32
F32R = mybir.dt.float32r
FP8 = mybir.dt.float8e4
BF16 = mybir.dt.bfloat16
FP8_SCALE = 64.0
AF = mybir.ActivationFunctionType

_COMPILED = None


def _build(reps=1, slab_first=True):
    # reps>1 repeats the whole computation inside one NEFF — used only by the
    # timing harness to amortize per-dispatch overhead; kernel() uses reps=1.
    nc = bacc.Bacc("TRN2", target_bir_lowering=False, debug=False)

    xT = nc.dram_tensor("xT", [IN, BL], BF16, kind="ExternalInput")
    hT = nc.dram_tensor("hT", [H, BL], BF16, kind="ExternalInput")
    # Weight slabs pre-tiled on host: [j_tile, p, k_tile, c] so each per-j
    # DMA reads 8KB contiguous per partition.
    wz = nc.dram_tensor("wz", [HT_, P, KT, P], BF16, kind="ExternalInput")
    wh = nc.dram_tensor("wh", [HT_, P, KT, P], BF16, kind="ExternalInput")
    # r-gate runs in fp8e4m3 DoubleRow (2x PE rate); weights pre-scaled by
    # FP8_SCALE on host, un-scaled in the sigmoid's activation scale.
    wr8 = nc.dram_tensor("wr8", [HT_, P, KT, P], FP8, kind="ExternalInput")
    xT8 = nc.dram_tensor("xT8", [IN, BL], FP8, kind="ExternalInput")
    hT8 = nc.dram_tensor("hT8", [H, BL], FP8, kind="ExternalInput")
    # Output projection pre-tiled: [p, j_tile, OUT]
    wo = nc.dram_tensor("wo", [P, HT_, OUT], BF16, kind="ExternalInput")
    bz = nc.dram_tensor("bz", [HT_, P], F32, kind="ExternalInput")
    br = nc.dram_tensor("br", [HT_, P], F32, kind="ExternalInput")
    bh = nc.dram_tensor("bh", [HT_, P], F32, kind="ExternalInput")
    bo = nc.dram_tensor("bo", [OT_, P], F32, kind="ExternalInput")

    hnewT = nc.dram_tensor("hnewT", [H, BL], BF16, kind="ExternalOutput")
    outT = nc.dram_tensor("outT", [OUT, BL], F32, kind="ExternalOutput")

    with tile.TileContext(nc) as tc:
        with (
            tc.tile_pool(name="resident", bufs=1) as resident,
            tc.tile_pool(name="wslab", bufs=6) as wslab,
            tc.tile_pool(name="w8", bufs=8) as w8pool,
            tc.tile_pool(name="wout", bufs=1) as wout,
            tc.tile_pool(name="acts", bufs=3) as acts,
            tc.tile_pool(name="dve", bufs=2) as dve,
            tc.tile_pool(name="psg", bufs=4, space="PSUM") as psg,
            tc.tile_pool(name="pso", bufs=4, space="PSUM") as pso,
        ):
            # ---- resident activations, [p, k_tile, b] transposed layout ----
            x_sb = resident.tile([P, XT_, BL], BF16)
            h_sb = resident.tile([P, HT_, BL], BF16)
            rh_sb = resident.tile([P, HT_, BL], BF16)    # r * h
            x8_sb = resident.tile([P, XT_, BL], FP8)
            h8_sb = resident.tile([P, HT_, BL], FP8)
            hn_sb = resident.tile([P, HT_, BL], BF16)    # new hidden

            bias_z = resident.tile([P, HT_], F32)
            bias_r = resident.tile([P, HT_], F32)
            bias_h = resident.tile([P, HT_], F32)
            bias_o = resident.tile([P, OT_], F32)

            def gate_rhs(k, bsl):
                src = x_sb if k < XT_ else h_sb
                return src[:, k % XT_, bsl]

            def cand_rhs(k, bsl):
                src = x_sb if k < XT_ else rh_sb
                return src[:, k % XT_, bsl]

            def emit_once():
                # Head DMA order is staged so the PE starts as early as
                # possible: first r-gate slab, then the b=0 halves of x/h
                # (enough for the first accumulation groups), then slab j=1,
                # then the b=1 halves. Everything else streams behind.
                # Stage the head DMA stream by when the PE needs it:
                # phase A's entire working set is tiny (4MB of fp8), so it
                # loads completely first; then phase B's b=0 f32r halves and
                # first slab pair; then the b=1 halves.
                preload = {}
                preloadB = {}
                pre0 = w8pool.tile([P, KT, P], FP8, tag="w8")
                preload[0] = pre0
                nc.sync.dma_start(out=preload[0], in_=wr8[0])
                nc.sync.dma_start(
                    out=x8_sb[:, :, :],
                    in_=xT8.rearrange("(t p) b -> p t b", p=P),
                )
                nc.sync.dma_start(
                    out=h8_sb[:, :, :],
                    in_=hT8.rearrange("(t p) b -> p t b", p=P),
                )
                nc.sync.dma_start(out=bias_r, in_=br.rearrange("t p -> p t"))
                for j in range(1, HT_):
                    prej = w8pool.tile([P, KT, P], FP8, tag="w8")
                    preload[j] = prej
                    nc.sync.dma_start(out=prej, in_=wr8[j])
                preB_z = wslab.tile([P, KT, P], BF16, tag="w")
                nc.sync.dma_start(out=preB_z, in_=wz[0])
                preB_h = wslab.tile([P, KT, P], BF16, tag="w")
                nc.sync.dma_start(out=preB_h, in_=wh[0])
                preloadB[0] = (preB_z, preB_h)
                xT3 = xT.rearrange("(t p) b -> p t b", p=P)
                hT3 = hT.rearrange("(t p) b -> p t b", p=P)
                for b in range(NB):
                    bsl = slice(b * 512, (b + 1) * 512)
                    nc.sync.dma_start(out=x_sb[:, :, bsl], in_=xT3[:, :, bsl])
                    nc.sync.dma_start(out=h_sb[:, :, bsl], in_=hT3[:, :, bsl])
                    if b == 0:
                        preB_z1 = wslab.tile([P, KT, P], BF16, tag="w")
                        nc.sync.dma_start(out=preB_z1, in_=wz[1])
                        preB_h1 = wslab.tile([P, KT, P], BF16, tag="w")
                        nc.sync.dma_start(out=preB_h1, in_=wh[1])
                        preloadB[1] = (preB_z1, preB_h1)
                nc.sync.dma_start(out=bias_z, in_=bz.rearrange("t p -> p t"))
                nc.sync.dma_start(out=bias_h, in_=bh.rearrange("t p -> p t"))
                nc.sync.dma_start(out=bias_o, in_=bo.rearrange("t p -> p t"))

                # ---- phase A: r gate (fp8 DoubleRow), rh = sigmoid(.)*h
                def gate_rhs8(t, bsl):
                    src8 = x8_sb if t < XT_ else h8_sb
                    return src8[:, t % XT_:t % XT_ + 2, bsl]

                for j in range(HT_):
                    if j in preload:
                        wr_sb = preload[j]
                    else:
                        wr_sb = w8pool.tile([P, KT, P], FP8, tag="w8")
                        nc.sync.dma_start(out=wr_sb, in_=wr8[j])
                    for b in range(NB):
                        bsl = slice(b * 512, (b + 1) * 512)
                        pr = psg.tile([P, 512], F32, tag="g")
                        for t in range(0, KT, 2):
                            nc.tensor.matmul(
                                pr[:, :],
                                wr_sb[:, t:t + 2, :],
                                gate_rhs8(t, bsl),
                                start=(t == 0),
                                stop=(t == KT - 2),
                                perf_mode=mybir.MatmulPerfMode.DoubleRow,
                            )
                        r_t = acts.tile([P, 512], F32, tag="act")
                        nc.scalar.activation(
                            out=r_t, in_=pr[:, :], func=AF.Sigmoid,
                            bias=bias_r[:, j:j + 1], scale=1.0 / FP8_SCALE,
                        )
                        nc.vector.tensor_mul(
                            rh_sb[:, j, bsl], r_t, h_sb[:, j, bsl]
                        )

                # ---- phase B: z gate + candidate + hidden update ----
                # Group order lags the b=1 groups of j=0/1 behind the b=0
                # groups so the PE never waits on the b=1 half of x/h, which
                # is still streaming when phase B starts.
                schedule = [(0, 0), (1, 0), (0, 1), (1, 1)]
                for j in range(2, HT_):
                    schedule += [(j, 0), (j, 1)]
                slabs = dict(preloadB)

                def emit_B_group(j, b):
                    bsl = slice(b * 512, (b + 1) * 512)
                    wz_sb, wh_sb = slabs[j]
                    pz = psg.tile([P, 512], F32, tag="g")
                    pc = psg.tile([P, 512], F32, tag="g")
                    for k in range(KT):
                        nc.tensor.matmul(
                            pz[:, :],
                            wz_sb[:, k, :],
                            gate_rhs(k, bsl),
                            start=(k == 0),
                            stop=(k == KT - 1),
                        )
                    for k in range(KT):
                        nc.tensor.matmul(
                            pc[:, :],
                            wh_sb[:, k, :],
                            cand_rhs(k, bsl),
                            start=(k == 0),
                            stop=(k == KT - 1),
                        )
                    z_t = acts.tile([P, 512], F32, tag="act")
                    nc.scalar.activation(
                        out=z_t, in_=pz[:, :], func=AF.Sigmoid,
                        bias=bias_z[:, j:j + 1],
                    )
                    ht_t = acts.tile([P, 512], F32, tag="act")
                    nc.scalar.activation(
                        out=ht_t, in_=pc[:, :], func=AF.Tanh,
                        bias=bias_h[:, j:j + 1],
                    )
                    # hnew = h + z * (h_tilde - h)
                    d_t = dve.tile([P, 512], F32, tag="d")
                    nc.vector.tensor_sub(d_t, ht_t, h_sb[:, j, bsl])
                    d2_t = dve.tile([P, 512], F32, tag="d")
                    nc.vector.tensor_mul(d2_t, d_t, z_t)
                    nc.vector.tensor_add(
                        hn_sb[:, j, bsl], d2_t, h_sb[:, j, bsl]
                    )
                    if b == 1:
                        nc.sync.dma_start(
                            out=hnewT[j * P:(j + 1) * P, :], in_=hn_sb[:, j, :]
                        )

                for j, b in schedule:
                    if j not in slabs:
                        wz_sb = wslab.tile([P, KT, P], BF16, tag="w")
                        nc.sync.dma_start(out=wz_sb, in_=wz[j])
                        wh_sb = wslab.tile([P, KT, P], BF16, tag="w")
                        nc.sync.dma_start(out=wh_sb, in_=wh[j])
                        slabs[j] = (wz_sb, wh_sb)
                    emit_B_group(j, b)

                # ---- phase C: output projection out^T = W_o @ hnew^T + b_o --
                wo_sb = wout.tile([P, HT_, OUT], BF16, tag="wo")
                nc.sync.dma_start(out=wo_sb, in_=wo[:, :, :])
                for b in range(NB):
                    bsl = slice(b * 512, (b + 1) * 512)
                    for o in range(OT_):
                        po = pso.tile([P, 512], F32, tag="o")
                        for j in range(HT_):
                            nc.tensor.matmul(
                                po[:, :],
                                wo_sb[:, j, o * P:(o + 1) * P],
                                hn_sb[:, j, bsl],
                                start=(j == 0),
                                stop=(j == HT_ - 1),
                            )
                        o_t = acts.tile([P, 512], F32, tag="act")
                        nc.scalar.activation(
                            out=o_t, in_=po[:, :], func=AF.Identity,
                            bias=bias_o[:, o:o + 1],
                        )
                        nc.sync.dma_start(
                            out=outT[o * P:(o + 1) * P, bsl], in_=o_t
                        )

            for _ in range(reps):
                emit_once()

    nc.compile()
    return nc


def _get_nc():
    global _COMPILED
    if _COMPILED is None:
        _COMPILED = _build()
    return _COMPILED


def _tile_gate_weight(w):
    # [H, K] -> [j_tile, p, k_tile, c]: slab j holds W.T[:, j*128:(j+1)*128]
    # with the contraction dim split into 16 partition-tiles.
    wT = np.ascontiguousarray(w.T)                        # [K, H]
    return np.ascontiguousarray(
        wT.reshape(KT, P, HT_, P).transpose(2, 1, 0, 3)   # [j, p, t, c]
    )


def _make_in_maps(inputs):
    fp8np = mybir.dt.np(FP8)
    bf16np = mybir.dt.np(BF16)
    xT = np.ascontiguousarray(np.asarray(inputs["x"], dtype=np.float32).T)
    hT = np.ascontiguousarray(np.asarray(inputs["hidden_state"], dtype=np.float32).T)
    xT8 = xT.astype(fp8np)
    hT8 = hT.astype(fp8np)
    xT = xT.astype(bf16np)
    hT = hT.astype(bf16np)

    wz_t = _tile_gate_weight(np.asarray(inputs["W_z"], dtype=np.float32)).astype(mybir.dt.np(BF16))
    wr8_t = (
        _tile_gate_weight(np.asarray(inputs["W_r"], dtype=np.float32)) * FP8_SCALE
    ).astype(fp8np)
    wh_t = _tile_gate_weight(np.asarray(inputs["W_h"], dtype=np.float32)).astype(mybir.dt.np(BF16))
    wo_t = np.ascontiguousarray(
        np.asarray(inputs["W_o"], dtype=np.float32).T
        .reshape(HT_, P, OUT).transpose(1, 0, 2)
    ).astype(mybir.dt.np(BF16))                                # [p, j, OUT]
    bz_t = np.ascontiguousarray(np.asarray(inputs["b_z"], np.float32).reshape(HT_, P))
    br_t = np.ascontiguousarray(np.asarray(inputs["b_r"], np.float32).reshape(HT_, P))
    bh_t = np.ascontiguousarray(np.asarray(inputs["b_h"], np.float32).reshape(HT_, P))
    bo_t = np.ascontiguousarray(np.asarray(inputs["b_o"], np.float32).reshape(OT_, P))

    in_maps = []
    for c in range(NCORES):
        csl = slice(c * BL, (c + 1) * BL)
        in_maps.append({
            "xT": np.ascontiguousarray(xT[:, csl]),
            "hT": np.ascontiguousarray(hT[:, csl]),
            "xT8": np.ascontiguousarray(xT8[:, csl]),
            "hT8": np.ascontiguousarray(hT8[:, csl]),
            "wz": wz_t, "wr8": wr8_t, "wh": wh_t, "wo": wo_t,
            "bz": bz_t, "br": br_t, "bh": bh_t, "bo": bo_t,
        })
    return in_maps


def kernel(x, hidden_state, W_z, b_z, W_r, b_r, W_h, b_h, W_o, b_o):
    nc = _get_nc()
    in_maps = _make_in_maps({
        "x": x, "hidden_state": hidden_state,
        "W_z": W_z, "b_z": b_z, "W_r": W_r, "b_r": b_r,
        "W_h": W_h, "b_h": b_h, "W_o": W_o, "b_o": b_o,
    })
    res = run_bass_kernel_spmd(nc, in_maps, list(range(NCORES)))

    output = np.empty((B, OUT), dtype=np.float32)
    hidden = np.empty((B, H), dtype=np.float32)
    for c in range(NCORES):
        csl = slice(c * BL, (c + 1) * BL)
        output[csl, :] = res.results[c]["outT"].T
        hidden[csl, :] = res.results[c]["hnewT"].T.astype(np.float32)
    return (output, hidden)


# revision 29
# speedup vs baseline: 7.5554x; 7.5554x over previous
"""Trainium2 Bass kernel for an ExGRU cell (GRU + output projection).

Reference computation (per batch row, B=8192, IN=1024, H=1024, OUT=512):
    xh      = concat(x, h)                  # [B, IN+H]
    z       = sigmoid(xh @ W_z.T + b_z)     # [B, H]
    r       = sigmoid(xh @ W_r.T + b_r)     # [B, H]
    xrh     = concat(x, r * h)
    h_tilde = tanh(xrh @ W_h.T + b_h)       # [B, H]
    hidden  = (1 - z) * h + z * h_tilde     # [B, H]
    output  = hidden @ W_o.T + b_o          # [B, OUT]
    return (output, hidden)

Strategy: data-parallel over the batch dim — each of the 8 cores gets 1024
batch rows; weights are replicated. Zero collectives. Everything on-device is
kept in a TRANSPOSED layout ([feature, batch]) so the contraction dim is always
on SBUF partitions and no on-device transposes are ever needed:

  - host pre-transposes x, h (batch → columns) and pre-tiles the weights,
  - gate outputs come out of the PE as z^T/r^T/h_tilde^T [H, B_loc] with the
    hidden dim on partitions, which is exactly the layout the candidate matmul
    and output projection need as their streaming operand,
  - host transposes the two outputs back after the gather.

Precision/speed mix (PE-bound kernel, so matmul dtype sets the roofline):
  - z-gate + candidate + output matmuls: bf16 (1 cycle/row at N=512 — same PE
    rate as fp32r but half the DMA bytes and SBUF), fp32 PSUM accumulation
    over K=2048 in 16 steps;
  - r-gate: fp8e4m3 with DoubleRow (2 k-tiles per matmul, 2x PE rate).
    Weights pre-scaled by 64 into fp8's normal range; the sigmoid's
    activation scale divides it back out. The r-gate tolerates fp8 because
    its error is damped by the sigmoid slope (<=0.25) and then again through
    the candidate matmul (~1e-3 contribution to hidden).
Measured end-to-end error vs the fp32 reference: ~2.9e-3 relative.

Phases: A) r-gate + rh=sigmoid()*h, B) z-gate + candidate + hidden update
(the b=1 groups of j=0/1 are lagged so the PE never waits on the second half
of x/h, which is still streaming when phase B starts), C) output projection.
Weight slabs stream from HBM double-buffered under the matmuls; x/h in both
precisions, rh, and hnew stay SBUF-resident; head DMAs are merged into few
large 3D-AP transfers because HWDGE dispatch (~0.6us each) would otherwise
dominate the warm-up.
"""

import numpy as np

import concourse.mybir as mybir
import concourse.tile as tile
from concourse import bacc
from concourse.bass_utils import run_bass_kernel_spmd

# Problem dims (hardcoded — kernel.py must be self-contained).
B, IN, H, OUT = 8192, 1024, 1024, 512
NCORES = 8
BL = B // NCORES      # 1024 local batch columns per core
K = IN + H            # 2048 gate contraction dim
P = 128               # SBUF partitions
KT = K // P           # 16 k-tiles per gate matmul
XT_ = IN // P         # 8 k-tiles of x
HT_ = H // P          # 8 k-tiles of h / j-tiles of the hidden dim
OT_ = OUT // P        # 4 o-tiles of the output projection
NB = BL // 512        # 2 batch chunks of 512 (fp32 moving-operand max)

F32 = mybir.dt.float32
F32R = mybir.dt.float32r
FP8 = mybir.dt.float8e4
BF16 = mybir.dt.bfloat16
FP8_SCALE = 64.0
AF = mybir.ActivationFunctionType

_COMPILED = None


def _build(reps=1, slab_first=True):
    # reps>1 repeats the whole computation inside one NEFF — used only by the
    # timing harness to amortize per-dispatch overhead; kernel() uses reps=1.
    nc = bacc.Bacc("TRN2", target_bir_lowering=False, debug=False)

    xT = nc.dram_tensor("xT", [IN, BL], BF16, kind="ExternalInput")
    hT = nc.dram_tensor("hT", [H, BL], BF16, kind="ExternalInput")
    # Weight slabs pre-tiled on host: [j_tile, p, k_tile, c] so each per-j
    # DMA reads 8KB contiguous per partition.
    wz = nc.dram_tensor("wz", [HT_, P, KT, P], BF16, kind="ExternalInput")
    wh = nc.dram_tensor("wh", [HT_, P, KT, P], BF16, kind="ExternalInput")
    # r-gate runs in fp8e4m3 DoubleRow (2x PE rate); weights pre-scaled by
    # FP8_SCALE on host, un-scaled in the sigmoid's activation scale.
    wr8 = nc.dram_tensor("wr8", [HT_, P, KT, P], FP8, kind="ExternalInput")
    xT8 = nc.dram_tensor("xT8", [IN, BL], FP8, kind="ExternalInput")
    hT8 = nc.dram_tensor("hT8", [H, BL], FP8, kind="ExternalInput")
    # Output projection pre-tiled: [p, j_tile, OUT]
    wo = nc.dram_tensor("wo", [P, HT_, OUT], BF16, kind="ExternalInput")
    bz = nc.dram_tensor("bz", [HT_, P], F32, kind="ExternalInput")
    br = nc.dram_tensor("br", [HT_, P], F32, kind="ExternalInput")
    bh = nc.dram_tensor("bh", [HT_, P], F32, kind="ExternalInput")
    bo = nc.dram_tensor("bo", [OT_, P], F32, kind="ExternalInput")

    hnewT = nc.dram_tensor("hnewT", [H, BL], BF16, kind="ExternalOutput")
    outT = nc.dram_tensor("outT", [OUT, BL], F32, kind="ExternalOutput")

    with tile.TileContext(nc) as tc:
        with (
            tc.tile_pool(name="resident", bufs=1) as resident,
            tc.tile_pool(name="wslab", bufs=6) as wslab,
            tc.tile_pool(name="w8", bufs=8) as w8pool,
            tc.tile_pool(name="wout", bufs=1) as wout,
            tc.tile_pool(name="acts", bufs=3) as acts,
            tc.tile_pool(name="dve", bufs=2) as dve,
            tc.tile_pool(name="psg", bufs=4, space="PSUM") as psg,
            tc.tile_pool(name="pso", bufs=4, space="PSUM") as pso,
        ):
            # ---- resident activations, [p, k_tile, b] transposed layout ----
            x_sb = resident.tile([P, XT_, BL], BF16)
            h_sb = resident.tile([P, HT_, BL], BF16)
            rh_sb = resident.tile([P, HT_, BL], BF16)    # r * h
            x8_sb = resident.tile([P, XT_, BL], FP8)
            h8_sb = resident.tile([P, HT_, BL], FP8)
            hn_sb = resident.tile([P, HT_, BL], BF16)    # new hidden

            bias_z = resident.tile([P, HT_], F32)
            bias_r = resident.tile([P, HT_], F32)
            bias_h = resident.tile([P, HT_], F32)
            bias_o = resident.tile([P, OT_], F32)

            def gate_rhs(k, bsl):
                src = x_sb if k < XT_ else h_sb
                return src[:, k % XT_, bsl]

            def cand_rhs(k, bsl):
                src = x_sb if k < XT_ else rh_sb
                return src[:, k % XT_, bsl]

            def emit_once():
                # Head DMA order is staged so the PE starts as early as
                # possible: first r-gate slab, then the b=0 halves of x/h
                # (enough for the first accumulation groups), then slab j=1,
                # then the b=1 halves. Everything else streams behind.
                # Stage the head DMA stream by when the PE needs it:
                # phase A's entire working set is tiny (4MB of fp8), so it
                # loads completely first; then phase B's b=0 f32r halves and
                # first slab pair; then the b=1 halves.
                preload = {}
                preloadB = {}
                pre0 = w8pool.tile([P, KT, P], FP8, tag="w8")
                preload[0] = pre0
                nc.sync.dma_start(out=preload[0], in_=wr8[0])
                xT83 = xT8.rearrange("(t p) b -> p t b", p=P)
                hT83 = hT8.rearrange("(t p) b -> p t b", p=P)
                half = XT_ // 2
                nc.sync.dma_start(out=x8_sb[:, :half, :], in_=xT83[:, :half, :])
                nc.sync.dma_start(out=x8_sb[:, half:, :], in_=xT83[:, half:, :])
                nc.sync.dma_start(out=h8_sb[:, :half, :], in_=hT83[:, :half, :])
                nc.sync.dma_start(out=h8_sb[:, half:, :], in_=hT83[:, half:, :])
                nc.sync.dma_start(out=bias_r, in_=br.rearrange("t p -> p t"))
                for j in range(1, HT_):
                    prej = w8pool.tile([P, KT, P], FP8, tag="w8")
                    preload[j] = prej
                    nc.sync.dma_start(out=prej, in_=wr8[j])
                preB_z = wslab.tile([P, KT, P], BF16, tag="w")
                nc.sync.dma_start(out=preB_z, in_=wz[0])
                preB_h = wslab.tile([P, KT, P], BF16, tag="w")
                nc.sync.dma_start(out=preB_h, in_=wh[0])
                preloadB[0] = (preB_z, preB_h)
                xT3 = xT.rearrange("(t p) b -> p t b", p=P)
                hT3 = hT.rearrange("(t p) b -> p t b", p=P)
                for b in range(NB):
                    bsl = slice(b * 512, (b + 1) * 512)
                    nc.sync.dma_start(out=x_sb[:, :, bsl], in_=xT3[:, :, bsl])
                    nc.sync.dma_start(out=h_sb[:, :, bsl], in_=hT3[:, :, bsl])
                    if b == 0:
                        preB_z1 = wslab.tile([P, KT, P], BF16, tag="w")
                        nc.sync.dma_start(out=preB_z1, in_=wz[1])
                        preB_h1 = wslab.tile([P, KT, P], BF16, tag="w")
                        nc.sync.dma_start(out=preB_h1, in_=wh[1])
                        preloadB[1] = (preB_z1, preB_h1)
                nc.sync.dma_start(out=bias_z, in_=bz.rearrange("t p -> p t"))
                nc.sync.dma_start(out=bias_h, in_=bh.rearrange("t p -> p t"))
                nc.sync.dma_start(out=bias_o, in_=bo.rearrange("t p -> p t"))

                # ---- phase A: r gate (fp8 DoubleRow), rh = sigmoid(.)*h
                def gate_rhs8(t, bsl):
                    src8 = x8_sb if t < XT_ else h8_sb
                    return src8[:, t % XT_:t % XT_ + 2, bsl]

                for j in range(HT_):
                    if j in preload:
                        wr_sb = preload[j]
                    else:
                        wr_sb = w8pool.tile([P, KT, P], FP8, tag="w8")
                        nc.sync.dma_start(out=wr_sb, in_=wr8[j])
                    for b in range(NB):
                        bsl = slice(b * 512, (b + 1) * 512)
                        pr = psg.tile([P, 512], F32, tag="g")
                        for t in range(0, KT, 2):
                            nc.tensor.matmul(
                                pr[:, :],
                                wr_sb[:, t:t + 2, :],
                                gate_rhs8(t, bsl),
                                start=(t == 0),
                                stop=(t == KT - 2),
                                perf_mode=mybir.MatmulPerfMode.DoubleRow,
                            )
                        r_t = acts.tile([P, 512], F32, tag="act")
                        nc.scalar.activation(
                            out=r_t, in_=pr[:, :], func=AF.Sigmoid,
                            bias=bias_r[:, j:j + 1], scale=1.0 / FP8_SCALE,
                        )
                        nc.vector.tensor_mul(
                            rh_sb[:, j, bsl], r_t, h_sb[:, j, bsl]
                        )

                # ---- phase B: z gate + candidate + hidden update ----
                # Group order lags the b=1 groups of j=0/1 behind the b=0
                # groups so the PE never waits on the b=1 half of x/h, which
                # is still streaming when phase B starts.
                schedule = [(0, 0), (1, 0), (0, 1), (1, 1)]
                for j in range(2, HT_):
                    schedule += [(j, 0), (j, 1)]
                slabs = dict(preloadB)

                def emit_B_group(j, b):
                    bsl = slice(b * 512, (b + 1) * 512)
                    wz_sb, wh_sb = slabs[j]
                    pz = psg.tile([P, 512], F32, tag="g")
                    pc = psg.tile([P, 512], F32, tag="g")
                    for k in range(KT):
                        nc.tensor.matmul(
                            pz[:, :],
                            wz_sb[:, k, :],
                            gate_rhs(k, bsl),
                            start=(k == 0),
                            stop=(k == KT - 1),
                        )
                    for k in range(KT):
                        nc.tensor.matmul(
                            pc[:, :],
                            wh_sb[:, k, :],
                            cand_rhs(k, bsl),
                            start=(k == 0),
                            stop=(k == KT - 1),
                        )
                    z_t = acts.tile([P, 512], F32, tag="act")
                    nc.scalar.activation(
                        out=z_t, in_=pz[:, :], func=AF.Sigmoid,
                        bias=bias_z[:, j:j + 1],
                    )
                    ht_t = acts.tile([P, 512], F32, tag="act")
                    nc.scalar.activation(
                        out=ht_t, in_=pc[:, :], func=AF.Tanh,
                        bias=bias_h[:, j:j + 1],
                    )
                    # hnew = h + z * (h_tilde - h)
                    d_t = dve.tile([P, 512], F32, tag="d")
                    nc.vector.tensor_sub(d_t, ht_t, h_sb[:, j, bsl])
                    d2_t = dve.tile([P, 512], F32, tag="d")
                    nc.vector.tensor_mul(d2_t, d_t, z_t)
                    nc.vector.tensor_add(
                        hn_sb[:, j, bsl], d2_t, h_sb[:, j, bsl]
                    )
                    if b == 1:
                        nc.sync.dma_start(
                            out=hnewT[j * P:(j + 1) * P, :], in_=hn_sb[:, j, :]
                        )

                for j, b in schedule:
                    if j not in slabs:
                        wz_sb = wslab.tile([P, KT, P], BF16, tag="w")
                        nc.sync.dma_start(out=wz_sb, in_=wz[j])
                        wh_sb = wslab.tile([P, KT, P], BF16, tag="w")
                        nc.sync.dma_start(out=wh_sb, in_=wh[j])
                        slabs[j] = (wz_sb, wh_sb)
                    emit_B_group(j, b)

                # ---- phase C: output projection out^T = W_o @ hnew^T + b_o --
                wo_sb = wout.tile([P, HT_, OUT], BF16, tag="wo")
                nc.sync.dma_start(out=wo_sb, in_=wo[:, :, :])
                for b in range(NB):
                    bsl = slice(b * 512, (b + 1) * 512)
                    for o in range(OT_):
                        po = pso.tile([P, 512], F32, tag="o")
                        for j in range(HT_):
                            nc.tensor.matmul(
                                po[:, :],
                                wo_sb[:, j, o * P:(o + 1) * P],
                                hn_sb[:, j, bsl],
                                start=(j == 0),
                                stop=(j == HT_ - 1),
                            )
                        o_t = acts.tile([P, 512], F32, tag="act")
                        nc.scalar.activation(
                            out=o_t, in_=po[:, :], func=AF.Identity,
                            bias=bias_o[:, o:o + 1],
                        )
                        nc.sync.dma_start(
                            out=outT[o * P:(o + 1) * P, bsl], in_=o_t
                        )

            for _ in range(reps):
                emit_once()

    nc.compile()
    return nc


def _get_nc():
    global _COMPILED
    if _COMPILED is None:
        _COMPILED = _build()
    return _COMPILED


def _tile_gate_weight(w):
    # [H, K] -> [j_tile, p, k_tile, c]: slab j holds W.T[:, j*128:(j+1)*128]
    # with the contraction dim split into 16 partition-tiles.
    wT = np.ascontiguousarray(w.T)                        # [K, H]
    return np.ascontiguousarray(
        wT.reshape(KT, P, HT_, P).transpose(2, 1, 0, 3)   # [j, p, t, c]
    )


def _make_in_maps(inputs):
    fp8np = mybir.dt.np(FP8)
    bf16np = mybir.dt.np(BF16)
    xT = np.ascontiguousarray(np.asarray(inputs["x"], dtype=np.float32).T)
    hT = np.ascontiguousarray(np.asarray(inputs["hidden_state"], dtype=np.float32).T)
    xT8 = xT.astype(fp8np)
    hT8 = hT.astype(fp8np)
    xT = xT.astype(bf16np)
    hT = hT.astype(bf16np)

    wz_t = _tile_gate_weight(np.asarray(inputs["W_z"], dtype=np.float32)).astype(mybir.dt.np(BF16))
    wr8_t = (
        _tile_gate_weight(np.asarray(inputs["W_r"], dtype=np.float32)) * FP8_SCALE
    ).astype(fp8np)
    wh_t = _tile_gate_weight(np.asarray(inputs["W_h"], dtype=np.float32)).astype(mybir.dt.np(BF16))
    wo_t = np.ascontiguousarray(
        np.asarray(inputs["W_o"], dtype=np.float32).T
        .reshape(HT_, P, OUT).transpose(1, 0, 2)
    ).astype(mybir.dt.np(BF16))                                # [p, j, OUT]
    bz_t = np.ascontiguousarray(np.asarray(inputs["b_z"], np.float32).reshape(HT_, P))
    br_t = np.ascontiguousarray(np.asarray(inputs["b_r"], np.float32).reshape(HT_, P))
    bh_t = np.ascontiguousarray(np.asarray(inputs["b_h"], np.float32).reshape(HT_, P))
    bo_t = np.ascontiguousarray(np.asarray(inputs["b_o"], np.float32).reshape(OT_, P))

    in_maps = []
    for c in range(NCORES):
        csl = slice(c * BL, (c + 1) * BL)
        in_maps.append({
            "xT": np.ascontiguousarray(xT[:, csl]),
            "hT": np.ascontiguousarray(hT[:, csl]),
            "xT8": np.ascontiguousarray(xT8[:, csl]),
            "hT8": np.ascontiguousarray(hT8[:, csl]),
            "wz": wz_t, "wr8": wr8_t, "wh": wh_t, "wo": wo_t,
            "bz": bz_t, "br": br_t, "bh": bh_t, "bo": bo_t,
        })
    return in_maps


def kernel(x, hidden_state, W_z, b_z, W_r, b_r, W_h, b_h, W_o, b_o):
    nc = _get_nc()
    in_maps = _make_in_maps({
        "x": x, "hidden_state": hidden_state,
        "W_z": W_z, "b_z": b_z, "W_r": W_r, "b_r": b_r,
        "W_h": W_h, "b_h": b_h, "W_o": W_o, "b_o": b_o,
    })
    res = run_bass_kernel_spmd(nc, in_maps, list(range(NCORES)))

    output = np.empty((B, OUT), dtype=np.float32)
    hidden = np.empty((B, H), dtype=np.float32)
    for c in range(NCORES):
        csl = slice(c * BL, (c + 1) * BL)
        output[csl, :] = res.results[c]["outT"].T
        hidden[csl, :] = res.results[c]["hnewT"].T.astype(np.float32)
    return (output, hidden)
